# revision 37
# baseline (speedup 1.0000x reference)
"""Trainium2 Bass kernel for: MLP (Linear -> BatchNorm1d(train) -> ReLU -> Linear)
followed by a per-bag segment softmax over ragged bags.

Contract: kernel(**inputs) takes FULL unsharded numpy inputs (keyed as in
setup_inputs()) and returns the FULL [N, 2] float32 output.

Strategy (8 NeuronCores, SPMD):
  - Host assigns whole bags to cores (LPT balance), pads each shard to CAP rows.
  - Features cast to bf16 on host; X^T tiles produced on-device by DMA-xbar
    transpose. h^T = W1^T @ X^T on the PE (bf16, fp32 PSUM accumulate).
  - BatchNorm batch stats are global across all rows: per-core partial
    sums/sumsq come from bn_stats on the PSUM (zero-padded rows contribute
    exact zeros), reduced with a tiny AllReduce across the 8 cores.
  - BN+ReLU fused into one ScalarE activation (relu(h*a + c)).
  - scores = hn @ W2 with hn tiles stationary -> scores [m,2] partition-major.
  - Segment softmax via per-bag 0/1 masks (built on host, fed as data) so the
    single SPMD program handles arbitrary ragged bag boundaries.
  - b2 is mathematically irrelevant (constant shift within each softmax group).
"""

import math

import numpy as np
import ml_dtypes

import concourse.bass as bass
import concourse.tile as tile
import concourse.mybir as mybir
import bass_isa
from concourse.vector_clock import ScopedClock
from concourse.masks import make_identity
from concourse.bass_utils import run_bass_kernel_spmd

F32 = mybir.dt.float32
BF16 = mybir.dt.bfloat16
AF = mybir.ActivationFunctionType
ALU = mybir.AluOpType
AX = mybir.AxisListType

N_CORES = 8
D_IN = 1024
D_HID = 512
D_OUT = 2
BN_EPS = 1e-5
BIG = 30000.0
DEBUG = False
BUILD_STAGE = "full"
LAST_RES = None
LAST_LAYOUTS = None
LAST_EXEC_NS = None
LAST_WALL_S = None

# ---------------------------------------------------------------------------
# Workaround: this walrus build only accepts one semaphore wait per
# instruction, but Tile emits instructions with several (the final drain and
# some DMA-transpose ops).  Post-pass: for any instruction with >1 waits,
# prepend same-engine NOPs each carrying one of the excess waits.
_MAX_WAITS = 1
_split_ctr = [0]


def _make_wait_nop(engine, waits):
    import bass_rust

    _split_ctr[0] += 1
    nop = bass_rust.InstNoOp(name=f"I-waitsplit-{_split_ctr[0]}", ins=[], outs=[])
    nop.engine = engine
    nop.sync_info = mybir.SyncInfo(on_update=[], on_wait=list(waits))
    return nop


def split_multiwait(nc, max_waits=_MAX_WAITS):
    for fn in nc.m.functions:
        for b in fn.blocks:
            insts = list(b.instructions)
            new, changed = [], False
            for inst in insts:
                si = inst.sync_info
                waits = list(si.on_wait) if (si is not None and si.on_wait) else []
                if len(waits) > max_waits:
                    changed = True
                    excess, keep = waits[:-max_waits], waits[-max_waits:]
                    for i in range(0, len(excess), max_waits):
                        new.append(
                            _make_wait_nop(inst.engine, excess[i : i + max_waits])
                        )
                    inst.sync_info = mybir.SyncInfo(
                        on_update=list(si.on_update) if si.on_update else [],
                        on_wait=keep,
                    )
                new.append(inst)
            if changed:
                b.instructions = new


# ---------------------------------------------------------------------------


def build_program(tc, io, cfg):
    """Emit the SPMD per-core program.

    io: dict of bass.APs: x [CAP, D_IN] bf16, w1 [D_IN, D_HID] bf16,
        w2 [128, NHB, D_OUT] bf16, b1v/gam/bet [128, NHB] f32,
        masks [n_slots, 128, 2*NT] f32, out [CAP, D_OUT] f32.
    cfg: dict with CAP, n_slots, n_total.
    """
    nc = tc.nc
    CAP = cfg["CAP"]
    n_slots = cfg["n_groups"]
    inv_n = 1.0 / float(cfg["n_total"])

    NKB = D_IN // 128            # 8 k-blocks
    NHB = D_HID // 128           # 4 hid-blocks
    NT = CAP // 128              # m-tiles
    NCH = CAP // 512             # 512-row chunks
    SC = min(1024, CAP)          # superchunk rows per transpose-DMA
    NSC = CAP // SC
    SUBS = SC // 512
    assert CAP % SC == 0 and CAP % 512 == 0 and 2 * NT <= 512

    x, w1, w2, gam, bet, masks, out = (
        io["x"], io["w1"], io["w2"], io["gam"], io["bet"],
        io["masks"], io["out"],
    )

    from contextlib import ExitStack

    ctx = ExitStack()
    consts = ctx.enter_context(tc.tile_pool(name="consts", bufs=1))
    xt_pool = ctx.enter_context(tc.tile_pool(name="xt", bufs=2))
    hn_pool = ctx.enter_context(tc.tile_pool(name="hn", bufs=2))
    small = ctx.enter_context(tc.tile_pool(name="small", bufs=1))
    obuf_pool = ctx.enter_context(tc.tile_pool(name="obuf", bufs=2))
    psum_h = ctx.enter_context(tc.tile_pool(name="psum_h", bufs=5, space="PSUM"))
    psum_s_pool = ctx.enter_context(tc.tile_pool(name="psum_s", bufs=1, space="PSUM"))
    psum_t_pool = ctx.enter_context(tc.tile_pool(name="psum_t", bufs=2, space="PSUM"))
    dram = ctx.enter_context(tc.tile_pool(name="dram", bufs=1, space="DRAM"))

    # ---- constants into SBUF ----
    # W1 k-blocks are loaded inside stage A's first superchunk, each just
    # ahead of its x^T transpose, so matmul k can start after ~2 transfers
    # instead of after the whole 1MB of W1.
    w1sb = consts.tile([128, NKB, D_HID], BF16)
    # w2/gamma/beta/masks are needed only after the collective — keep them
    # off the sync queue so they don't delay the first x^T transposes.
    w2sb = consts.tile([128, NHB, D_OUT], BF16)
    nc.gpsimd.dma_start(out=w2sb[:], in_=w2[:])
    gamsb = consts.tile([128, NHB], F32)
    nc.gpsimd.dma_start(out=gamsb[:], in_=gam[:])
    betsb = consts.tile([128, NHB], F32)
    nc.gpsimd.dma_start(out=betsb[:], in_=bet[:])
    msb = consts.tile([128, n_slots, 2 * NT], F32)
    for s in range(n_slots):
        nc.gpsimd.dma_start(out=msb[:, s, :], in_=masks[s])
    eps_t = consts.tile([128, 1], F32)
    nc.vector.memset(eps_t[:], BN_EPS)
    idf = consts.tile([128, 128], F32)
    make_identity(nc, idf[:])
    ones_k = consts.tile([128, 1], F32)     # [128,1] of ones (cross-part sums)
    nc.vector.memset(ones_k[:], 1.0)
    ones_m = consts.tile([1, 128], F32)     # [1,128] of ones (broadcasts)
    nc.vector.memset(ones_m[:], 1.0)

    # big persistent h^T store (bf16): [128, NHB, CAP]
    hsb = consts.tile([128, NHB, CAP], BF16)
    statsbuf = consts.tile([128, NHB, NCH, 6], F32)

    def emit_stats_collective(ch0, ch1, tag):
        """bn_aggr over chunk range [ch0, ch1) -> local (sum, sumsq)
        -> AllReduce across cores; returns the [128, 8] global tile."""
        mv = small.tile([128, NHB, 2], F32, name=f"mv{tag}")
        for hb in range(NHB):
            nc.vector.bn_aggr(out=mv[:, hb, :], in_=statsbuf[:, hb, ch0:ch1, :])
        cnt = float((ch1 - ch0) * 512)
        s8 = small.tile([128, 8], F32, name=f"s8{tag}")
        means = mv[:, :, 0]
        varis = mv[:, :, 1]
        nc.vector.tensor_scalar_mul(out=s8[:, 0:NHB], in0=means, scalar1=cnt)
        tmp4 = small.tile([128, NHB], F32, name=f"tmp4{tag}")
        nc.vector.tensor_mul(out=tmp4[:], in0=means, in1=means)
        nc.vector.tensor_add(out=tmp4[:], in0=tmp4[:], in1=varis)
        nc.vector.tensor_scalar_mul(out=s8[:, NHB : 2 * NHB], in0=tmp4[:], scalar1=cnt)
        cin = dram.tile([128, 8], F32, name=f"cin{tag}")
        cout = dram.tile([128, 8], F32, name=f"cout{tag}")
        nc.gpsimd.dma_start(out=cin[:], in_=s8[:])
        nc.gpsimd.collective_compute(
            "AllReduce",
            ALU.add,
            replica_groups=[list(range(N_CORES))],
            ins=[cin.opt()],
            outs=[cout.opt()],
        )
        g = small.tile([128, 8], F32, name=f"g{tag}")
        nc.gpsimd.dma_start(out=g[:], in_=cout[:])
        return g

    # NOTE: collective_compute acts as a cross-engine ordering point in
    # this runtime, so a collective can NOT overlap subsequently-emitted
    # engine work — a single stats collective at the end of stage A is
    # the best placement.

    # ---- Stage A: h^T = W1^T @ X^T (no bias: b1 cancels in BN), stats ----
    for sc in range(NSC):
        xts = []
        for k in range(NKB):
            if sc == 0:
                nc.sync.dma_start(
                    out=w1sb[:, k, :], in_=w1[k * 128 : (k + 1) * 128, :]
                )
            xk = xt_pool.tile([128, SC], BF16, tag=f"xt{k}")
            nc.sync.dma_start(
                out=xk[:],
                in_=x[sc * SC : (sc + 1) * SC, k * 128 : (k + 1) * 128],
                transpose=True,
            )
            xts.append(xk)
        for sub in range(SUBS):
            c = sc * SUBS + sub
            for hb in range(NHB):
                ph = psum_h.tile([128, 512], F32, tag="ph", name=f"ph_{c}_{hb}")
                for k in range(NKB):
                    nc.tensor.matmul(
                        ph[:],
                        w1sb[:, k, hb * 128 : (hb + 1) * 128],
                        xts[k][:, sub * 512 : (sub + 1) * 512],
                        start=(k == 0),
                        stop=(k == NKB - 1),
                    )
                nc.scalar.copy(
                    out=hsb[:, hb, c * 512 : (c + 1) * 512], in_=ph[:]
                )
                # partial stats of pre-bias h (pads contribute exact zeros)
                nc.vector.bn_stats(out=statsbuf[:, hb, c, :], in_=ph[:])

    if BUILD_STAGE == "A":
        ctx.close()
        return

    # ---- stats sums -> AllReduce -> BN affine coefficients ----
    g8 = emit_stats_collective(0, NCH, "G")

    meanp = small.tile([128, NHB], F32)
    nc.vector.tensor_scalar_mul(out=meanp[:], in0=g8[:, 0:NHB], scalar1=inv_n)
    varg = small.tile([128, NHB], F32)
    nc.vector.tensor_scalar_mul(out=varg[:], in0=g8[:, NHB : 2 * NHB], scalar1=inv_n)
    m2 = small.tile([128, NHB], F32)
    nc.vector.tensor_mul(out=m2[:], in0=meanp[:], in1=meanp[:])
    nc.vector.tensor_sub(out=varg[:], in0=varg[:], in1=m2[:])
    stdv = small.tile([128, NHB], F32)
    nc.scalar.activation(out=stdv[:], in_=varg[:], func=AF.Sqrt, bias=eps_t[:], scale=1.0)
    rstd = small.tile([128, NHB], F32)
    nc.vector.reciprocal(out=rstd[:], in_=stdv[:])
    if cfg.get("gamma_one", False):
        av = rstd
    else:
        av = small.tile([128, NHB], F32)
        nc.vector.tensor_mul(out=av[:], in0=gamsb[:], in1=rstd[:])
    if not (cfg.get("fold_relu", False) and cfg.get("beta_zero", False)):
        c2 = small.tile([128, NHB], F32)
        mtmp = small.tile([128, NHB], F32)
        nc.vector.tensor_mul(out=mtmp[:], in0=meanp[:], in1=av[:])
        nc.vector.tensor_sub(out=c2[:], in0=betsb[:], in1=mtmp[:])

    if BUILD_STAGE == "B":
        ctx.close()
        return

    # ---- Stage C: normalize + relu, then scores = hn @ W2 ([m,2] major) ----
    psum_s = psum_s_pool.tile([128, 2 * NT], F32)
    if cfg.get("fold_relu", False):
        # gamma > 0 everywhere: relu(a*h + c) = a * relu(h + c/a).  Fold the
        # per-channel scale a into W2 (one tiny op per block) and normalize
        # IN-PLACE on the h store with a single fused subtract+relu per span.
        bia = small.tile([128, NHB], F32)      # c/a
        if cfg.get("beta_zero", False):
            # beta == 0: c/a = -mean, skip the reciprocal round-trip
            nc.vector.tensor_scalar_mul(out=bia[:], in0=meanp[:], scalar1=-1.0)
        else:
            winv = small.tile([128, NHB], F32)
            nc.vector.reciprocal(out=winv[:], in_=av[:])
            nc.vector.tensor_mul(out=bia[:], in0=c2[:], in1=winv[:])
        w2f = small.tile([128, NHB, D_OUT], BF16)
        for hb in range(NHB):
            nc.vector.tensor_scalar_mul(
                out=w2f[:, hb, :], in0=w2sb[:, hb, :], scalar1=av[:, hb : hb + 1]
            )
        # Interleave normalize (scalar+vector) with the score matmuls
        # (tensor) in row groups so the engines pipeline.
        GROUP = 2048 if CAP % 2048 == 0 else 512
        TPG = GROUP // 128
        for g in range(CAP // GROUP):
            s0 = g * GROUP
            for hb in range(NHB):
                seg = hsb[:, hb, s0 : s0 + GROUP]
                if hb == 0:
                    nc.scalar.activation(
                        out=seg, in_=seg, func=AF.Relu,
                        bias=bia[:, hb : hb + 1], scale=1.0,
                    )
                else:
                    nc.vector.tensor_scalar(
                        out=seg, in0=seg,
                        scalar1=bia[:, hb : hb + 1], scalar2=0.0,
                        op0=ALU.add, op1=ALU.max,
                    )
            for mt in range(TPG):
                t = g * TPG + mt
                for hb in range(NHB):
                    nc.tensor.matmul(
                        psum_s[:, 2 * t : 2 * t + 2],
                        hsb[:, hb, t * 128 : (t + 1) * 128],
                        w2f[:, hb, :],
                        start=(hb == 0),
                        stop=(hb == NHB - 1),
                    )
    else:
        for c in range(NCH):
            hn = hn_pool.tile([128, NHB, 512], BF16, tag="hn")
            for hb in range(NHB):
                if hb < NHB // 4:
                    # ScalarE: fused relu(h*a + c)
                    nc.scalar.activation(
                        out=hn[:, hb, :],
                        in_=hsb[:, hb, c * 512 : (c + 1) * 512],
                        func=AF.Relu,
                        bias=c2[:, hb : hb + 1],
                        scale=av[:, hb : hb + 1],
                    )
                else:
                    # VectorE: affine (4x-mode bf16 tensor_scalar) then relu
                    nc.vector.tensor_scalar(
                        out=hn[:, hb, :],
                        in0=hsb[:, hb, c * 512 : (c + 1) * 512],
                        scalar1=av[:, hb : hb + 1],
                        scalar2=c2[:, hb : hb + 1],
                        op0=ALU.mult,
                        op1=ALU.add,
                    )
                    nc.vector.tensor_relu(out=hn[:, hb, :], in_=hn[:, hb, :])
            for mt in range(4):
                t = c * 4 + mt
                for hb in range(NHB):
                    nc.tensor.matmul(
                        psum_s[:, 2 * t : 2 * t + 2],
                        hn[:, hb, mt * 128 : (mt + 1) * 128],
                        w2sb[:, hb, :],
                        start=(hb == 0),
                        stop=(hb == NHB - 1),
                    )

    if BUILD_STAGE == "C":
        ctx.close()
        return

    # ---- Stage D: masked segment softmax on [128, 2*NT] ----
    # BatchNorm bounds |scores| to O(1), so exp() cannot overflow in f32
    # and the usual max-subtraction pass is skipped entirely: E = exp(S)
    # straight out of PSUM.  Cross-partition sums for all n_slots bags are
    # batched through single ones-matmul hops.
    T2 = small.tile([128, 2 * NT], F32)
    E = small.tile([128, 2 * NT], F32)
    nc.scalar.activation(out=E[:], in_=psum_s[:], func=AF.Exp)
    # per-slot per-partition sums, packed into [128, n_slots]
    PS4 = small.tile([128, n_slots], F32)
    for s in range(n_slots):
        nc.vector.tensor_mul(out=T2[:], in0=E[:], in1=msb[:, s, :])
        nc.vector.tensor_reduce(
            out=PS4[:, s : s + 1], in_=T2[:], axis=AX.X, op=ALU.add
        )
    # cross-partition sums of all slots in one ones-matmul -> [1, n_slots]
    pq = psum_t_pool.tile([128, 128], F32, tag="pt")
    nc.tensor.matmul(pq[:1, 0:n_slots], ones_k[:], PS4[:], start=True, stop=True)
    srow = small.tile([1, n_slots], F32)
    nc.vector.tensor_copy(out=srow[:], in_=pq[:1, 0:n_slots])
    nc.vector.tensor_scalar_max(out=srow[:], in0=srow[:], scalar1=1e-30)
    nc.vector.reciprocal(out=srow[:], in_=srow[:])
    ptc = psum_t_pool.tile([128, 128], F32, tag="pt")
    nc.tensor.matmul(ptc[:, 0:n_slots], ones_m[:], srow[:], start=True, stop=True)
    AI = small.tile([128, n_slots], F32)
    nc.vector.tensor_copy(out=AI[:], in_=ptc[:, 0:n_slots])
    IV = small.tile([128, 2 * NT], F32)
    nc.vector.tensor_scalar_mul(out=IV[:], in0=msb[:, 0, :], scalar1=AI[:, 0:1])
    for s in range(1, n_slots):
        nc.vector.tensor_scalar_mul(out=T2[:], in0=msb[:, s, :], scalar1=AI[:, s : s + 1])
        nc.vector.tensor_add(out=IV[:], in0=IV[:], in1=T2[:])
    OUTt = small.tile([128, 2 * NT], F32)
    nc.vector.tensor_mul(out=OUTt[:], in0=E[:], in1=IV[:])

    if "dbg" in io:
        d = io["dbg"]
        nc.sync.dma_start(out=d["S"], in_=S[:])
        nc.sync.dma_start(out=d["E"], in_=E[:])
        nc.sync.dma_start(out=d["MV"], in_=MV[:])
        nc.sync.dma_start(out=d["IV"], in_=IV[:])
        nc.sync.dma_start(out=d["av"], in_=av[:])
        nc.sync.dma_start(out=d["c2"], in_=c2[:])
        nc.sync.dma_start(out=d["meanp"], in_=meanp[:])
        nc.sync.dma_start(out=d["varg"], in_=varg[:])
        nc.sync.dma_start(out=d["g8"], in_=g8[:])
        nc.sync.dma_start(out=d["s8"], in_=s8[:])
        nc.sync.dma_start(out=d["stats"], in_=statsbuf[:])
        nc.gpsimd.dma_start(out=d["h0"], in_=hsb[:, :, 0:512])

    # ---- transpose to m-contiguous layout and DMA out ----
    out3 = out.rearrange("(t p) j -> t p j", p=128)  # [NT, 128, 2]
    OUT3 = OUTt[:].rearrange("p (t j) -> p t j", j=D_OUT)
    for t0 in range(0, NT, 128):
        ntg = min(128, NT - t0)
        ob = obuf_pool.tile([128, 128, D_OUT], F32, tag="ob")
        for j in range(D_OUT):
            pt = psum_t_pool.tile([128, 128], F32, tag="pt")
            nc.tensor.transpose(
                pt[:ntg, :], OUT3[:, t0 : t0 + ntg, j], idf[:]
            )
            nc.scalar.copy(out=ob[:ntg, :, j], in_=pt[:ntg, :])
        nc.sync.dma_start(out=out3[t0 : t0 + ntg], in_=ob[:ntg])

    ctx.close()


# ---------------------------------------------------------------------------
# Host-side orchestration
# ---------------------------------------------------------------------------


def _assign_bags(bag_sizes):
    """LPT-assign whole bags to cores; returns per-core list of bag ids."""
    order = np.argsort(-bag_sizes, kind="stable")
    loads = [0] * N_CORES
    assign = [[] for _ in range(N_CORES)]
    for b in order:
        c = int(np.argmin(loads))
        assign[c].append(int(b))
        loads[c] += int(bag_sizes[b])
    for c in range(N_CORES):
        assign[c].sort()
    return assign


def prepare(features, W1, b1, gamma, beta, W2, b2, bag_sizes, reps=1):
    n_total, d_in = features.shape
    assert d_in == D_IN
    bag_sizes = np.asarray(bag_sizes, dtype=np.int64)
    bag_off = np.concatenate([[0], np.cumsum(bag_sizes)])
    assert bag_off[-1] == n_total

    assign = _assign_bags(bag_sizes)
    n_slots = max(1, max(len(a) for a in assign))
    max_load = max(int(sum(bag_sizes[b] for b in a)) for a in assign)
    CAP = max(1024, ((max_load + 1023) // 1024) * 1024)
    NT = CAP // 128

    xbf = np.asarray(features, dtype=ml_dtypes.bfloat16)
    w1bf = np.asarray(W1, dtype=ml_dtypes.bfloat16)
    # w2 prearranged [128, NHB, D_OUT]
    w2bf = (
        np.asarray(W2, dtype=ml_dtypes.bfloat16)
        .reshape(D_HID // 128, 128, D_OUT)
        .transpose(1, 0, 2)
        .copy()
    )

    def vec128(v):
        return (
            np.asarray(v, dtype=np.float32).reshape(D_HID // 128, 128).T.copy()
        )

    gamv, betv = vec128(gamma), vec128(beta)

    in_maps = []
    layouts = []  # per core: list of (bag_id, row_offset, size)
    for c in range(N_CORES):
        xs = np.zeros((CAP, D_IN), dtype=ml_dtypes.bfloat16)
        masks = np.zeros((n_slots * D_OUT, 128, 2 * NT), dtype=np.float32)
        off = 0
        lay = []
        for s, b in enumerate(assign[c]):
            sz = int(bag_sizes[b])
            xs[off : off + sz] = xbf[bag_off[b] : bag_off[b] + sz]
            rows = np.arange(off, off + sz)
            t, p = rows // 128, rows % 128
            for j in range(D_OUT):
                masks[s * D_OUT + j, p, 2 * t + j] = 1.0
            lay.append((b, off, sz))
            off += sz
        layouts.append(lay)
        in_maps.append(
            {
                "x": xs,
                "w1": w1bf,
                "w2": w2bf,
                "gam": gamv,
                "bet": betv,
                "masks": masks,
            }
        )

    nc = bass.Bass("TRN2", target_bir_lowering=False, debug=False, num_devices=N_CORES)
    io = {
        "x": nc.dram_tensor("x", [CAP, D_IN], BF16, kind="ExternalInput").ap(),
        "w1": nc.dram_tensor("w1", [D_IN, D_HID], BF16, kind="ExternalInput").ap(),
        "w2": nc.dram_tensor("w2", [128, D_HID // 128, D_OUT], BF16, kind="ExternalInput").ap(),
        "gam": nc.dram_tensor("gam", [128, D_HID // 128], F32, kind="ExternalInput").ap(),
        "bet": nc.dram_tensor("bet", [128, D_HID // 128], F32, kind="ExternalInput").ap(),
        "masks": nc.dram_tensor("masks", [n_slots * D_OUT, 128, 2 * NT], F32, kind="ExternalInput").ap(),
        "out": nc.dram_tensor("out", [CAP, D_OUT], F32, kind="ExternalOutput").ap(),
    }
    gam_arr = np.asarray(gamma, dtype=np.float64)
    bet_arr = np.asarray(beta, dtype=np.float64)
    fold_relu = bool((gam_arr > 1e-6).all())
    cfg = {"CAP": CAP, "n_groups": n_slots * D_OUT, "n_total": n_total,
           "fold_relu": fold_relu,
           "gamma_one": bool((gam_arr == 1.0).all()),
           "beta_zero": bool((bet_arr == 0.0).all())}
    if DEBUG:
        NHB = D_HID // 128
        io["dbg"] = {
            "S": nc.dram_tensor("dS", [128, 2 * NT], F32, kind="ExternalOutput").ap(),
            "E": nc.dram_tensor("dE", [128, 2 * NT], F32, kind="ExternalOutput").ap(),
            "MV": nc.dram_tensor("dMV", [128, 2 * NT], F32, kind="ExternalOutput").ap(),
            "IV": nc.dram_tensor("dIV", [128, 2 * NT], F32, kind="ExternalOutput").ap(),
            "av": nc.dram_tensor("dav", [128, NHB], F32, kind="ExternalOutput").ap(),
            "c2": nc.dram_tensor("dc2", [128, NHB], F32, kind="ExternalOutput").ap(),
            "meanp": nc.dram_tensor("dmeanp", [128, NHB], F32, kind="ExternalOutput").ap(),
            "varg": nc.dram_tensor("dvarg", [128, NHB], F32, kind="ExternalOutput").ap(),
            "g8": nc.dram_tensor("dg8", [128, 8], F32, kind="ExternalOutput").ap(),
            "s8": nc.dram_tensor("ds8", [128, 8], F32, kind="ExternalOutput").ap(),
            "stats": nc.dram_tensor("dstats", [128, D_HID // 128, CAP // 512, 6], F32, kind="ExternalOutput").ap(),
            "h0": nc.dram_tensor("dh0", [128, NHB, 512], F32, kind="ExternalOutput").ap(),
        }
    with tile.TileContext(nc) as tc:
        for _ in range(reps):
            build_program(tc, io, cfg)
    split_multiwait(nc)
    return nc, in_maps, layouts, bag_off, n_total


def kernel(features, W1, b1, gamma, beta, W2, b2, bag_sizes):
    nc, in_maps, layouts, bag_off, n_total = prepare(
        features, W1, b1, gamma, beta, W2, b2, bag_sizes
    )

    import os as _os
    import time as _time

    _t0 = _time.time()
    _tr = bool(_os.environ.get("KTRACE"))
    _kw = {}
    if _tr:
        _kw["trace"] = True
        _td = _os.environ.get("KTRACE_DIR")
        if _td:
            global _KTRACE_CTR
            try:
                _KTRACE_CTR += 1
            except NameError:
                _KTRACE_CTR = 0
            _td = f"{_td}/run{_KTRACE_CTR}"
            _os.makedirs(_td, exist_ok=True)
            _kw["tmpdir"] = _td
    res = run_bass_kernel_spmd(nc, in_maps, core_ids=list(range(N_CORES)), **_kw)
    global LAST_RES, LAST_LAYOUTS, LAST_EXEC_NS, LAST_WALL_S
    LAST_WALL_S = _time.time() - _t0
    LAST_EXEC_NS = res.exec_time_ns
    LAST_RES, LAST_LAYOUTS = res, layouts

    out_full = np.empty((n_total, D_OUT), dtype=np.float32)
    for c in range(N_CORES):
        oc = res.results[c]["out"]
        for b, off, sz in layouts[c]:
            out_full[bag_off[b] : bag_off[b] + sz] = oc[off : off + sz]
    return out_full



# revision 39
# speedup vs baseline: 1.0920x; 1.0920x over previous
"""Trainium2 Bass kernel for: MLP (Linear -> BatchNorm1d(train) -> ReLU -> Linear)
followed by a per-bag segment softmax over ragged bags.

Contract: kernel(**inputs) takes FULL unsharded numpy inputs (keyed as in
setup_inputs()) and returns the FULL [N, 2] float32 output.

Strategy (8 NeuronCores, SPMD):
  - Host assigns whole bags to cores (LPT balance), pads each shard to CAP rows.
  - Features cast to bf16 on host; X^T tiles produced on-device by DMA-xbar
    transpose. h^T = W1^T @ X^T on the PE (bf16, fp32 PSUM accumulate).
  - BatchNorm batch stats are global across all rows: per-core partial
    sums/sumsq come from bn_stats on the PSUM (zero-padded rows contribute
    exact zeros), reduced with a tiny AllReduce across the 8 cores.
  - BN+ReLU fused into one ScalarE activation (relu(h*a + c)).
  - scores = hn @ W2 with hn tiles stationary -> scores [m,2] partition-major.
  - Segment softmax via per-bag 0/1 masks (built on host, fed as data) so the
    single SPMD program handles arbitrary ragged bag boundaries.
  - b2 is mathematically irrelevant (constant shift within each softmax group).
"""

import math

import numpy as np
import ml_dtypes

import concourse.bass as bass
import concourse.tile as tile
import concourse.mybir as mybir
import bass_isa
from concourse.vector_clock import ScopedClock
from concourse.masks import make_identity
from concourse.bass_utils import run_bass_kernel_spmd

F32 = mybir.dt.float32
BF16 = mybir.dt.bfloat16
AF = mybir.ActivationFunctionType
ALU = mybir.AluOpType
AX = mybir.AxisListType

N_CORES = 8
D_IN = 1024
D_HID = 512
D_OUT = 2
BN_EPS = 1e-5
BIG = 30000.0
DEBUG = False
BUILD_STAGE = "full"
LAST_RES = None
LAST_LAYOUTS = None
LAST_EXEC_NS = None
LAST_WALL_S = None

# ---------------------------------------------------------------------------
# Workaround: this walrus build only accepts one semaphore wait per
# instruction, but Tile emits instructions with several (the final drain and
# some DMA-transpose ops).  Post-pass: for any instruction with >1 waits,
# prepend same-engine NOPs each carrying one of the excess waits.
_MAX_WAITS = 1
_split_ctr = [0]


def _make_wait_nop(engine, waits):
    import bass_rust

    _split_ctr[0] += 1
    nop = bass_rust.InstNoOp(name=f"I-waitsplit-{_split_ctr[0]}", ins=[], outs=[])
    nop.engine = engine
    nop.sync_info = mybir.SyncInfo(on_update=[], on_wait=list(waits))
    return nop


def split_multiwait(nc, max_waits=_MAX_WAITS):
    for fn in nc.m.functions:
        for b in fn.blocks:
            insts = list(b.instructions)
            new, changed = [], False
            for inst in insts:
                si = inst.sync_info
                waits = list(si.on_wait) if (si is not None and si.on_wait) else []
                if len(waits) > max_waits:
                    changed = True
                    excess, keep = waits[:-max_waits], waits[-max_waits:]
                    for i in range(0, len(excess), max_waits):
                        new.append(
                            _make_wait_nop(inst.engine, excess[i : i + max_waits])
                        )
                    inst.sync_info = mybir.SyncInfo(
                        on_update=list(si.on_update) if si.on_update else [],
                        on_wait=keep,
                    )
                new.append(inst)
            if changed:
                b.instructions = new


# ---------------------------------------------------------------------------


def build_program(tc, io, cfg):
    """Emit the SPMD per-core program.

    io: dict of bass.APs: x [CAP, D_IN] bf16, w1 [D_IN, D_HID] bf16,
        w2 [128, NHB, D_OUT] bf16, b1v/gam/bet [128, NHB] f32,
        masks [n_slots, 128, 2*NT] f32, out [CAP, D_OUT] f32.
    cfg: dict with CAP, n_slots, n_total.
    """
    nc = tc.nc
    CAP = cfg["CAP"]
    n_slots = cfg["n_groups"]
    inv_n = 1.0 / float(cfg["n_total"])

    NKB = D_IN // 128            # 8 k-blocks
    NHB = D_HID // 128           # 4 hid-blocks
    NT = CAP // 128              # m-tiles
    NCH = CAP // 512             # 512-row chunks
    SC = min(1024, CAP)          # superchunk rows per transpose-DMA
    NSC = CAP // SC
    SUBS = SC // 512
    assert CAP % SC == 0 and CAP % 512 == 0 and 2 * NT <= 512

    x, w1, w2, gam, bet, masks, out = (
        io["x"], io["w1"], io["w2"], io["gam"], io["bet"],
        io["masks"], io["out"],
    )

    from contextlib import ExitStack

    ctx = ExitStack()
    consts = ctx.enter_context(tc.tile_pool(name="consts", bufs=1))
    xt_pool = ctx.enter_context(tc.tile_pool(name="xt", bufs=2))
    hn_pool = ctx.enter_context(tc.tile_pool(name="hn", bufs=2))
    small = ctx.enter_context(tc.tile_pool(name="small", bufs=1))
    obuf_pool = ctx.enter_context(tc.tile_pool(name="obuf", bufs=2))
    psum_h = ctx.enter_context(tc.tile_pool(name="psum_h", bufs=5, space="PSUM"))
    psum_s_pool = ctx.enter_context(tc.tile_pool(name="psum_s", bufs=1, space="PSUM"))
    psum_t_pool = ctx.enter_context(tc.tile_pool(name="psum_t", bufs=2, space="PSUM"))
    dram = ctx.enter_context(tc.tile_pool(name="dram", bufs=1, space="DRAM"))

    # ---- constants into SBUF ----
    w1sb = consts.tile([128, NKB, D_HID], BF16)
    for k in range(NKB):
        nc.sync.dma_start(out=w1sb[:, k, :], in_=w1[k * 128 : (k + 1) * 128, :])
    # w2/gamma/beta/masks are needed only after the collective — keep them
    # off the sync queue so they don't delay the first x^T transposes.
    w2sb = consts.tile([128, NHB, D_OUT], BF16)
    nc.gpsimd.dma_start(out=w2sb[:], in_=w2[:])
    gamsb = consts.tile([128, NHB], F32)
    nc.gpsimd.dma_start(out=gamsb[:], in_=gam[:])
    betsb = consts.tile([128, NHB], F32)
    nc.gpsimd.dma_start(out=betsb[:], in_=bet[:])
    msb = consts.tile([128, n_slots, 2 * NT], F32)
    for s in range(n_slots):
        nc.gpsimd.dma_start(out=msb[:, s, :], in_=masks[s])
    eps_t = consts.tile([128, 1], F32)
    nc.vector.memset(eps_t[:], BN_EPS)
    idf = consts.tile([128, 128], F32)
    make_identity(nc, idf[:])
    ones_k = consts.tile([128, 1], F32)     # [128,1] of ones (cross-part sums)
    nc.vector.memset(ones_k[:], 1.0)
    ones_m = consts.tile([1, 128], F32)     # [1,128] of ones (broadcasts)
    nc.vector.memset(ones_m[:], 1.0)

    # big persistent h^T store (bf16): [128, NHB, CAP]
    hsb = consts.tile([128, NHB, CAP], BF16)
    statsbuf = consts.tile([128, NHB, NCH, 6], F32)

    def emit_stats_collective(ch0, ch1, tag):
        """bn_aggr over chunk range [ch0, ch1) -> local (sum, sumsq)
        -> AllReduce across cores; returns the [128, 8] global tile."""
        mv = small.tile([128, NHB, 2], F32, name=f"mv{tag}")
        for hb in range(NHB):
            nc.vector.bn_aggr(out=mv[:, hb, :], in_=statsbuf[:, hb, ch0:ch1, :])
        cnt = float((ch1 - ch0) * 512)
        s8 = small.tile([128, 8], F32, name=f"s8{tag}")
        means = mv[:, :, 0]
        varis = mv[:, :, 1]
        nc.vector.tensor_scalar_mul(out=s8[:, 0:NHB], in0=means, scalar1=cnt)
        tmp4 = small.tile([128, NHB], F32, name=f"tmp4{tag}")
        nc.vector.tensor_mul(out=tmp4[:], in0=means, in1=means)
        nc.vector.tensor_add(out=tmp4[:], in0=tmp4[:], in1=varis)
        nc.vector.tensor_scalar_mul(out=s8[:, NHB : 2 * NHB], in0=tmp4[:], scalar1=cnt)
        cin = dram.tile([128, 8], F32, name=f"cin{tag}")
        cout = dram.tile([128, 8], F32, name=f"cout{tag}")
        nc.gpsimd.dma_start(out=cin[:], in_=s8[:])
        nc.gpsimd.collective_compute(
            "AllReduce",
            ALU.add,
            replica_groups=[list(range(N_CORES))],
            ins=[cin.opt()],
            outs=[cout.opt()],
        )
        g = small.tile([128, 8], F32, name=f"g{tag}")
        nc.gpsimd.dma_start(out=g[:], in_=cout[:])
        return g

    # NOTE: collective_compute acts as a cross-engine ordering point in
    # this runtime, so a collective can NOT overlap subsequently-emitted
    # engine work — a single stats collective at the end of stage A is
    # the best placement.

    # ---- Stage A: h^T = W1^T @ X^T (no bias: b1 cancels in BN), stats ----
    for sc in range(NSC):
        xts = []
        for k in range(NKB):
            xk = xt_pool.tile([128, SC], BF16, tag=f"xt{k}")
            nc.sync.dma_start(
                out=xk[:],
                in_=x[sc * SC : (sc + 1) * SC, k * 128 : (k + 1) * 128],
                transpose=True,
            )
            xts.append(xk)
        for sub in range(SUBS):
            c = sc * SUBS + sub
            for hb in range(NHB):
                ph = psum_h.tile([128, 512], F32, tag="ph", name=f"ph_{c}_{hb}")
                for k in range(NKB):
                    nc.tensor.matmul(
                        ph[:],
                        w1sb[:, k, hb * 128 : (hb + 1) * 128],
                        xts[k][:, sub * 512 : (sub + 1) * 512],
                        start=(k == 0),
                        stop=(k == NKB - 1),
                    )
                nc.scalar.copy(
                    out=hsb[:, hb, c * 512 : (c + 1) * 512], in_=ph[:]
                )
                # partial stats of pre-bias h (pads contribute exact zeros)
                nc.vector.bn_stats(out=statsbuf[:, hb, c, :], in_=ph[:])

    if BUILD_STAGE == "A":
        ctx.close()
        return

    # ---- stats sums -> AllReduce -> BN affine coefficients ----
    g8 = emit_stats_collective(0, NCH, "G")

    meanp = small.tile([128, NHB], F32)
    nc.vector.tensor_scalar_mul(out=meanp[:], in0=g8[:, 0:NHB], scalar1=inv_n)
    varg = small.tile([128, NHB], F32)
    nc.vector.tensor_scalar_mul(out=varg[:], in0=g8[:, NHB : 2 * NHB], scalar1=inv_n)
    m2 = small.tile([128, NHB], F32)
    nc.vector.tensor_mul(out=m2[:], in0=meanp[:], in1=meanp[:])
    nc.vector.tensor_sub(out=varg[:], in0=varg[:], in1=m2[:])
    stdv = small.tile([128, NHB], F32)
    nc.scalar.activation(out=stdv[:], in_=varg[:], func=AF.Sqrt, bias=eps_t[:], scale=1.0)
    rstd = small.tile([128, NHB], F32)
    nc.vector.reciprocal(out=rstd[:], in_=stdv[:])
    if cfg.get("gamma_one", False):
        av = rstd
    else:
        av = small.tile([128, NHB], F32)
        nc.vector.tensor_mul(out=av[:], in0=gamsb[:], in1=rstd[:])
    if not (cfg.get("fold_relu", False) and cfg.get("beta_zero", False)):
        c2 = small.tile([128, NHB], F32)
        mtmp = small.tile([128, NHB], F32)
        nc.vector.tensor_mul(out=mtmp[:], in0=meanp[:], in1=av[:])
        nc.vector.tensor_sub(out=c2[:], in0=betsb[:], in1=mtmp[:])

    if BUILD_STAGE == "B":
        ctx.close()
        return

    # ---- Stage C: normalize + relu, then scores = hn @ W2 ([m,2] major) ----
    psum_s = psum_s_pool.tile([128, 2 * NT], F32)
    if cfg.get("fold_relu", False):
        # gamma > 0 everywhere: relu(a*h + c) = a * relu(h + c/a).  Fold the
        # per-channel scale a into W2 (one tiny op per block) and normalize
        # IN-PLACE on the h store with a single fused subtract+relu per span.
        bia = small.tile([128, NHB], F32)      # c/a
        if cfg.get("beta_zero", False):
            # beta == 0: c/a = -mean, skip the reciprocal round-trip
            nc.vector.tensor_scalar_mul(out=bia[:], in0=meanp[:], scalar1=-1.0)
        else:
            winv = small.tile([128, NHB], F32)
            nc.vector.reciprocal(out=winv[:], in_=av[:])
            nc.vector.tensor_mul(out=bia[:], in0=c2[:], in1=winv[:])
        w2f = small.tile([128, NHB, D_OUT], BF16)
        for hb in range(NHB):
            nc.vector.tensor_scalar_mul(
                out=w2f[:, hb, :], in0=w2sb[:, hb, :], scalar1=av[:, hb : hb + 1]
            )
        # Interleave normalize (scalar+vector) with the score matmuls
        # (tensor) in row groups so the engines pipeline.
        GROUP = 2048 if CAP % 2048 == 0 else 512
        TPG = GROUP // 128
        for g in range(CAP // GROUP):
            s0 = g * GROUP
            for hb in range(NHB):
                seg = hsb[:, hb, s0 : s0 + GROUP]
                if hb == 0:
                    nc.scalar.activation(
                        out=seg, in_=seg, func=AF.Relu,
                        bias=bia[:, hb : hb + 1], scale=1.0,
                    )
                else:
                    nc.vector.tensor_scalar(
                        out=seg, in0=seg,
                        scalar1=bia[:, hb : hb + 1], scalar2=0.0,
                        op0=ALU.add, op1=ALU.max,
                    )
            for mt in range(TPG):
                t = g * TPG + mt
                for hb in range(NHB):
                    nc.tensor.matmul(
                        psum_s[:, 2 * t : 2 * t + 2],
                        hsb[:, hb, t * 128 : (t + 1) * 128],
                        w2f[:, hb, :],
                        start=(hb == 0),
                        stop=(hb == NHB - 1),
                    )
    else:
        for c in range(NCH):
            hn = hn_pool.tile([128, NHB, 512], BF16, tag="hn")
            for hb in range(NHB):
                if hb < NHB // 4:
                    # ScalarE: fused relu(h*a + c)
                    nc.scalar.activation(
                        out=hn[:, hb, :],
                        in_=hsb[:, hb, c * 512 : (c + 1) * 512],
                        func=AF.Relu,
                        bias=c2[:, hb : hb + 1],
                        scale=av[:, hb : hb + 1],
                    )
                else:
                    # VectorE: affine (4x-mode bf16 tensor_scalar) then relu
                    nc.vector.tensor_scalar(
                        out=hn[:, hb, :],
                        in0=hsb[:, hb, c * 512 : (c + 1) * 512],
                        scalar1=av[:, hb : hb + 1],
                        scalar2=c2[:, hb : hb + 1],
                        op0=ALU.mult,
                        op1=ALU.add,
                    )
                    nc.vector.tensor_relu(out=hn[:, hb, :], in_=hn[:, hb, :])
            for mt in range(4):
                t = c * 4 + mt
                for hb in range(NHB):
                    nc.tensor.matmul(
                        psum_s[:, 2 * t : 2 * t + 2],
                        hn[:, hb, mt * 128 : (mt + 1) * 128],
                        w2sb[:, hb, :],
                        start=(hb == 0),
                        stop=(hb == NHB - 1),
                    )

    if BUILD_STAGE == "C":
        ctx.close()
        return

    # ---- Stage D: masked segment softmax on [128, 2*NT] ----
    # BatchNorm bounds |scores| to O(1), so exp() cannot overflow in f32
    # and the usual max-subtraction pass is skipped entirely: E = exp(S)
    # straight out of PSUM.  Cross-partition sums for all n_slots bags are
    # batched through single ones-matmul hops.
    T2 = small.tile([128, 2 * NT], F32)
    E = small.tile([128, 2 * NT], F32)
    nc.scalar.activation(out=E[:], in_=psum_s[:], func=AF.Exp)
    # per-slot per-partition sums, packed into [128, n_slots]
    PS4 = small.tile([128, n_slots], F32)
    for s in range(n_slots):
        nc.vector.tensor_mul(out=T2[:], in0=E[:], in1=msb[:, s, :])
        nc.vector.tensor_reduce(
            out=PS4[:, s : s + 1], in_=T2[:], axis=AX.X, op=ALU.add
        )
    # cross-partition sums of all slots in one ones-matmul -> [1, n_slots]
    pq = psum_t_pool.tile([128, 128], F32, tag="pt")
    nc.tensor.matmul(pq[:1, 0:n_slots], ones_k[:], PS4[:], start=True, stop=True)
    srow = small.tile([1, n_slots], F32)
    nc.vector.tensor_copy(out=srow[:], in_=pq[:1, 0:n_slots])
    nc.vector.tensor_scalar_max(out=srow[:], in0=srow[:], scalar1=1e-30)
    nc.vector.reciprocal(out=srow[:], in_=srow[:])
    ptc = psum_t_pool.tile([128, 128], F32, tag="pt")
    nc.tensor.matmul(ptc[:, 0:n_slots], ones_m[:], srow[:], start=True, stop=True)
    AI = small.tile([128, n_slots], F32)
    nc.vector.tensor_copy(out=AI[:], in_=ptc[:, 0:n_slots])
    IV = small.tile([128, 2 * NT], F32)
    nc.vector.tensor_scalar_mul(out=IV[:], in0=msb[:, 0, :], scalar1=AI[:, 0:1])
    for s in range(1, n_slots):
        nc.vector.tensor_scalar_mul(out=T2[:], in0=msb[:, s, :], scalar1=AI[:, s : s + 1])
        nc.vector.tensor_add(out=IV[:], in0=IV[:], in1=T2[:])
    OUTt = small.tile([128, 2 * NT], F32)
    nc.vector.tensor_mul(out=OUTt[:], in0=E[:], in1=IV[:])

    if "dbg" in io:
        d = io["dbg"]
        nc.sync.dma_start(out=d["S"], in_=S[:])
        nc.sync.dma_start(out=d["E"], in_=E[:])
        nc.sync.dma_start(out=d["MV"], in_=MV[:])
        nc.sync.dma_start(out=d["IV"], in_=IV[:])
        nc.sync.dma_start(out=d["av"], in_=av[:])
        nc.sync.dma_start(out=d["c2"], in_=c2[:])
        nc.sync.dma_start(out=d["meanp"], in_=meanp[:])
        nc.sync.dma_start(out=d["varg"], in_=varg[:])
        nc.sync.dma_start(out=d["g8"], in_=g8[:])
        nc.sync.dma_start(out=d["s8"], in_=s8[:])
        nc.sync.dma_start(out=d["stats"], in_=statsbuf[:])
        nc.gpsimd.dma_start(out=d["h0"], in_=hsb[:, :, 0:512])

    # ---- transpose to m-contiguous layout and DMA out ----
    out3 = out.rearrange("(t p) j -> t p j", p=128)  # [NT, 128, 2]
    OUT3 = OUTt[:].rearrange("p (t j) -> p t j", j=D_OUT)
    for t0 in range(0, NT, 128):
        ntg = min(128, NT - t0)
        ob = obuf_pool.tile([128, 128, D_OUT], F32, tag="ob")
        for j in range(D_OUT):
            pt = psum_t_pool.tile([128, 128], F32, tag="pt")
            nc.tensor.transpose(
                pt[:ntg, :], OUT3[:, t0 : t0 + ntg, j], idf[:]
            )
            nc.scalar.copy(out=ob[:ntg, :, j], in_=pt[:ntg, :])
        nc.sync.dma_start(out=out3[t0 : t0 + ntg], in_=ob[:ntg])

    ctx.close()


# ---------------------------------------------------------------------------
# Host-side orchestration
# ---------------------------------------------------------------------------


def _assign_bags(bag_sizes):
    """LPT-assign whole bags to cores; returns per-core list of bag ids."""
    order = np.argsort(-bag_sizes, kind="stable")
    loads = [0] * N_CORES
    assign = [[] for _ in range(N_CORES)]
    for b in order:
        c = int(np.argmin(loads))
        assign[c].append(int(b))
        loads[c] += int(bag_sizes[b])
    for c in range(N_CORES):
        assign[c].sort()
    return assign


def prepare(features, W1, b1, gamma, beta, W2, b2, bag_sizes, reps=1):
    n_total, d_in = features.shape
    assert d_in == D_IN
    bag_sizes = np.asarray(bag_sizes, dtype=np.int64)
    bag_off = np.concatenate([[0], np.cumsum(bag_sizes)])
    assert bag_off[-1] == n_total

    assign = _assign_bags(bag_sizes)
    n_slots = max(1, max(len(a) for a in assign))
    max_load = max(int(sum(bag_sizes[b] for b in a)) for a in assign)
    CAP = max(1024, ((max_load + 1023) // 1024) * 1024)
    NT = CAP // 128

    xbf = np.asarray(features, dtype=ml_dtypes.bfloat16)
    w1bf = np.asarray(W1, dtype=ml_dtypes.bfloat16)
    # w2 prearranged [128, NHB, D_OUT]
    w2bf = (
        np.asarray(W2, dtype=ml_dtypes.bfloat16)
        .reshape(D_HID // 128, 128, D_OUT)
        .transpose(1, 0, 2)
        .copy()
    )

    def vec128(v):
        return (
            np.asarray(v, dtype=np.float32).reshape(D_HID // 128, 128).T.copy()
        )

    gamv, betv = vec128(gamma), vec128(beta)

    in_maps = []
    layouts = []  # per core: list of (bag_id, row_offset, size)
    for c in range(N_CORES):
        xs = np.zeros((CAP, D_IN), dtype=ml_dtypes.bfloat16)
        masks = np.zeros((n_slots * D_OUT, 128, 2 * NT), dtype=np.float32)
        off = 0
        lay = []
        for s, b in enumerate(assign[c]):
            sz = int(bag_sizes[b])
            xs[off : off + sz] = xbf[bag_off[b] : bag_off[b] + sz]
            rows = np.arange(off, off + sz)
            t, p = rows // 128, rows % 128
            for j in range(D_OUT):
                masks[s * D_OUT + j, p, 2 * t + j] = 1.0
            lay.append((b, off, sz))
            off += sz
        layouts.append(lay)
        in_maps.append(
            {
                "x": xs,
                "w1": w1bf,
                "w2": w2bf,
                "gam": gamv,
                "bet": betv,
                "masks": masks,
            }
        )

    nc = bass.Bass("TRN2", target_bir_lowering=False, debug=False, num_devices=N_CORES)
    io = {
        "x": nc.dram_tensor("x", [CAP, D_IN], BF16, kind="ExternalInput").ap(),
        "w1": nc.dram_tensor("w1", [D_IN, D_HID], BF16, kind="ExternalInput").ap(),
        "w2": nc.dram_tensor("w2", [128, D_HID // 128, D_OUT], BF16, kind="ExternalInput").ap(),
        "gam": nc.dram_tensor("gam", [128, D_HID // 128], F32, kind="ExternalInput").ap(),
        "bet": nc.dram_tensor("bet", [128, D_HID // 128], F32, kind="ExternalInput").ap(),
        "masks": nc.dram_tensor("masks", [n_slots * D_OUT, 128, 2 * NT], F32, kind="ExternalInput").ap(),
        "out": nc.dram_tensor("out", [CAP, D_OUT], F32, kind="ExternalOutput").ap(),
    }
    gam_arr = np.asarray(gamma, dtype=np.float64)
    bet_arr = np.asarray(beta, dtype=np.float64)
    fold_relu = bool((gam_arr > 1e-6).all())
    cfg = {"CAP": CAP, "n_groups": n_slots * D_OUT, "n_total": n_total,
           "fold_relu": fold_relu,
           "gamma_one": bool((gam_arr == 1.0).all()),
           "beta_zero": bool((bet_arr == 0.0).all())}
    if DEBUG:
        NHB = D_HID // 128
        io["dbg"] = {
            "S": nc.dram_tensor("dS", [128, 2 * NT], F32, kind="ExternalOutput").ap(),
            "E": nc.dram_tensor("dE", [128, 2 * NT], F32, kind="ExternalOutput").ap(),
            "MV": nc.dram_tensor("dMV", [128, 2 * NT], F32, kind="ExternalOutput").ap(),
            "IV": nc.dram_tensor("dIV", [128, 2 * NT], F32, kind="ExternalOutput").ap(),
            "av": nc.dram_tensor("dav", [128, NHB], F32, kind="ExternalOutput").ap(),
            "c2": nc.dram_tensor("dc2", [128, NHB], F32, kind="ExternalOutput").ap(),
            "meanp": nc.dram_tensor("dmeanp", [128, NHB], F32, kind="ExternalOutput").ap(),
            "varg": nc.dram_tensor("dvarg", [128, NHB], F32, kind="ExternalOutput").ap(),
            "g8": nc.dram_tensor("dg8", [128, 8], F32, kind="ExternalOutput").ap(),
            "s8": nc.dram_tensor("ds8", [128, 8], F32, kind="ExternalOutput").ap(),
            "stats": nc.dram_tensor("dstats", [128, D_HID // 128, CAP // 512, 6], F32, kind="ExternalOutput").ap(),
            "h0": nc.dram_tensor("dh0", [128, NHB, 512], F32, kind="ExternalOutput").ap(),
        }
    with tile.TileContext(nc) as tc:
        for _ in range(reps):
            build_program(tc, io, cfg)
    split_multiwait(nc)
    return nc, in_maps, layouts, bag_off, n_total


def kernel(features, W1, b1, gamma, beta, W2, b2, bag_sizes):
    nc, in_maps, layouts, bag_off, n_total = prepare(
        features, W1, b1, gamma, beta, W2, b2, bag_sizes
    )

    import os as _os
    import time as _time

    _t0 = _time.time()
    _tr = bool(_os.environ.get("KTRACE"))
    _kw = {}
    if _tr:
        _kw["trace"] = True
        _td = _os.environ.get("KTRACE_DIR")
        if _td:
            global _KTRACE_CTR
            try:
                _KTRACE_CTR += 1
            except NameError:
                _KTRACE_CTR = 0
            _td = f"{_td}/run{_KTRACE_CTR}"
            _os.makedirs(_td, exist_ok=True)
            _kw["tmpdir"] = _td
    res = run_bass_kernel_spmd(nc, in_maps, core_ids=list(range(N_CORES)), **_kw)
    global LAST_RES, LAST_LAYOUTS, LAST_EXEC_NS, LAST_WALL_S
    LAST_WALL_S = _time.time() - _t0
    LAST_EXEC_NS = res.exec_time_ns
    LAST_RES, LAST_LAYOUTS = res, layouts

    out_full = np.empty((n_total, D_OUT), dtype=np.float32)
    for c in range(N_CORES):
        oc = res.results[c]["out"]
        for b, off, sz in layouts[c]:
            out_full[bag_off[b] : bag_off[b] + sz] = oc[off : off + sz]
    return out_full

